# revision 40
# baseline (speedup 1.0000x reference)
"""FlowNet correlation kernel for Trainium2 (8 NeuronCores, batch-parallel).

Problem: out[b, d, y, x] = (1/C) * sum_c i1[b,c,y,x] * pad(i2)[b,c,y+dy,x+dx]
  B=8, C=256, H=48, W=64, pad=20, displacements dy,dx in {-20..20 step 2}
  (21x21 = 441), output [8, 441, 48, 64] fp32.

Strategy (per core, one batch element):
  Displacement stride 2 => the problem splits into 4 independent polyphase
  subproblems (y-parity sy, x-parity sx), each a dense +-10 correlation on a
  24x32 quarter image. Block output pixels 8 sub-rows x 16 sub-cols
  (M = 128): each block's displacement band is a 28x36 window of the padded
  polyphase i2, but only the in-bounds part of it is ever computed — the
  matmuls are clipped to the real window rows AND columns (pad rows/cols of
  the band are statically zero and are filled in by the host). Per block
  that leaves real-rows x 26 band columns: 18x26=468 (fits one PSUM bank,
  single fp16 matmul per k-tile) for the top/bottom row-blocks, 24x26=624
  (split 12+12 over two banks) for the middle one. fp16 matmuls run at full
  PE rate with fp32 accumulation; the two 128-channel k-tiles accumulate in
  PSUM. The 1/C scale is applied during the PSUM->SBUF copy (fp16 band,
  split DVE/scalar), and the bands of each (s, yb) block pair are dumped
  with one contiguous big-packet DMA.

  Host-side prep (part of the sharding step, not device time): inputs are
  cast to fp16 and re-laid out per core — i1 pre-polyphased and pre-blocked
  [C, 4, 6, 128] so it DMAs directly into the stationary matmul layout, i2
  polyphased unpadded [C, 2, 2, 24, 32]. Input DMAs are chunked and ordered
  so the first block's operands land first, and the PE clock is pre-warmed
  with zero matmuls while inputs stream. The host scatters the compact
  bands back into zero-initialized full windows and extracts each pixel's
  21x21 displacement window (a strided view + copy) to assemble
  [441, 48, 64] fp32.

  History: an earlier design gathered the 441 displacement values per pixel
  on-device with diagonal-access-pattern DMAs — its 84-byte packets
  (64512/core) were DMA-packet-rate-bound. fp32 matmuls run at 1/4 PE rate
  (and float32r requires explicitly rounded producers), hence fp16. DMA
  queues sustain ~25 GB/s each (~400 GB/s/core aggregate) for packets
  >= ~1.5 KB, so all device DMAs are big contiguous runs and every byte
  shipped is real data.
"""

import numpy as np

C = 256
H, W = 48, 64
ND = 21                      # displacements per axis
D = ND * ND                  # 441
SUB_H, SUB_W = H // 2, W // 2          # 24, 32
BH, BW = 8, 16               # block = 8 x 16 output pixels (one polyphase)
WRH, WRW = BH + 20, BW + 20  # 28 x 36 full band window per block
NYB, NXB = SUB_H // BH, SUB_W // BW    # 3, 2
NCC = BW + 10                # 26 real band columns per block
ROWS = [18, 24, 18]          # real band rows per yb  (of 28)
RLO = [10, 2, 0]             # first real band row per yb
CLO = [10, 0]                # first real band column per xb
NPAIR = 4 * NYB              # 12 dumped (s, yb) pair records
PAIRW = 2 * max(ROWS) * NCC  # 1248 fp16 cols per pair record (max)

_CACHE = {}


def _build():
    import concourse.bacc as bacc
    import concourse.mybir as mybir
    from concourse.tile import TileContext

    f32 = mybir.dt.float32
    f16 = mybir.dt.float16

    nc = bacc.Bacc("TRN2", target_bir_lowering=False, debug=False)
    # i1: [C, s, blk, m] fp16, pre-polyphased/pre-blocked on host
    i1_t = nc.dram_tensor("i1", [C, 4 * NYB * NXB * 128], f16, kind="ExternalInput")
    # i2: [C, sy, sx, 24, 32] fp16, polyphased, unpadded
    i2_t = nc.dram_tensor("i2", [C, 4 * SUB_H * SUB_W], f16, kind="ExternalInput")
    od_t = nc.dram_tensor("od", [NPAIR, 128, PAIRW], f16, kind="ExternalOutput")

    inv_c = 1.0 / C

    with TileContext(nc) as tc:
        with (
            tc.tile_pool(name="inp", bufs=1) as inp_pool,
            tc.tile_pool(name="band", bufs=6) as band_pool,
            tc.tile_pool(name="ps", bufs=8, space="PSUM") as ps_pool,
        ):
            i1s_sb = [
                inp_pool.tile(
                    [128, 4 * NYB * NXB * 128], f16, name=f"i1k{k}", tag=f"i1k{k}"
                )
                for k in range(2)
            ]
            i2_sb = [
                inp_pool.tile(
                    [128, 4 * SUB_H * SUB_W], f16, name=f"i2k{k}", tag=f"i2k{k}"
                )
                for k in range(2)
            ]
            # [c, (sy sx), row, col] unpadded polyphase
            i2v = [
                t[:].rearrange("c (s r w) -> c s r w", s=4, r=SUB_H) for t in i2_sb
            ]

            # warm up the PE clock (HAM p-state ramps over ~3us of activity)
            # with zeros x zeros matmuls while the inputs stream in
            zt = inp_pool.tile([128, 504], f16, name="warmz", tag="warmz")
            nc.gpsimd.memset(zt[:], 0.0)
            wps = ps_pool.tile([128, 512], f32, name="ps")
            for _ in range(8):
                nc.tensor.matmul(
                    wps[:, 0:504], lhsT=zt[:, 0:128], rhs=zt[:], start=True,
                    stop=True,
                )

            # chunked input loads on the sync HWDGE queue, ordered to
            # unblock the first blocks first (scalar stays copy-only: it is
            # in the PSUM-drain path and must not fall behind the PE)
            for s in range(4):
                for k in range(2):
                    cs = slice(128 * k, 128 * (k + 1))
                    if s % 2 == 0:  # i1 in two chunks per k (s01, s23)
                        nc.sync.dma_start(
                            out=i1s_sb[k][:, 1536 * (s // 2) : 1536 * (s // 2 + 1)],
                            in_=i1_t.ap()[cs, 1536 * (s // 2) : 1536 * (s // 2 + 1)],
                        )
                    nc.sync.dma_start(
                        out=i2v[k][:, s],
                        in_=i2_t.ap()[cs, 768 * s : 768 * (s + 1)],
                    )
            # pair dumps follow on the same queue: its head blocks on each
            # dump's copy-completion semaphores, so output streaming tracks
            # the compute pipeline pair by pair

            for s in range(4):
                for yb in range(NYB):
                    n = ROWS[yb] * NCC          # real band cols per block
                    # row spans per matmul: single for 18-row blocks, 12+12
                    # across two PSUM banks for the 24-row middle block
                    if ROWS[yb] <= 19:
                        spans = [(0, ROWS[yb])]
                    else:
                        h = ROWS[yb] // 2
                        spans = [(0, h), (h, ROWS[yb])]
                    r_base = BH * yb + RLO[yb] - 10  # first real i2 row
                    band2 = band_pool.tile([128, PAIRW], f16, name="band")
                    # one single-bank PSUM tile per (xb, span): bufs=8 keeps
                    # a full s-phase of banks in flight
                    pss = [
                        [ps_pool.tile([128, 512], f32, name="ps") for _ in spans]
                        for _ in range(NXB)
                    ]
                    # k outermost across the xb pair: all k0 matmuls run
                    # before any k1 data is needed, hiding the k1 input
                    # chunks' DMA-completion latency
                    for k in range(2):
                        for xb in range(NXB):
                            blk = s * NYB * NXB + yb * NXB + xb
                            c0 = CLO[xb] + BW * xb - 10  # first real i2 col
                            lhs = i1s_sb[k][:, 128 * blk : 128 * (blk + 1)]
                            for h, (lo, hi) in enumerate(spans):
                                rh = i2v[k][
                                    :,
                                    s,
                                    r_base + lo : r_base + hi,
                                    c0 : c0 + NCC,
                                ]
                                nc.tensor.matmul(
                                    pss[xb][h][:, 0 : NCC * (hi - lo)],
                                    lhsT=lhs,
                                    rhs=rh,
                                    start=(k == 0),
                                    stop=(k == 1),
                                )
                            if k == 1:
                                # compact to SBUF with the 1/C scale (fp16
                                # band); one copy per bank, split DVE/scalar
                                engs = (
                                    [nc.vector.tensor_scalar_mul, nc.scalar.mul]
                                    if xb == 0
                                    else [nc.scalar.mul, nc.vector.tensor_scalar_mul]
                                )
                                for h, (lo, hi) in enumerate(spans):
                                    engs[h % len(engs)](
                                        band2[
                                            :, xb * n + NCC * lo : xb * n + NCC * hi
                                        ],
                                        pss[xb][h][:, 0 : NCC * (hi - lo)],
                                        inv_c,
                                    )
                    p = s * NYB + yb
                    nc.sync.dma_start(
                        out=od_t.ap()[p][:, 0 : 2 * n], in_=band2[:, 0 : 2 * n]
                    )

    nc.compile()
    return nc


def _get_program():
    if "nc" not in _CACHE:
        _CACHE["nc"] = _build()
    return _CACHE["nc"]


def _prep_i1(x: np.ndarray) -> np.ndarray:
    """[C, H, W] fp32 -> [C, 4*6*128] fp16 pre-polyphased + pre-blocked."""
    # [c, sy, sx, yb, xb, ry, rx] <- x[c, 16yb+2ry+sy, 32xb+2rx+sx]
    v = x.reshape(C, NYB, BH, 2, NXB, BW, 2)
    v = v.transpose(0, 3, 6, 1, 4, 2, 5)  # c, sy, sx, yb, xb, ry, rx
    return np.ascontiguousarray(v, dtype=np.float16).reshape(C, -1)


def _prep_i2(x: np.ndarray) -> np.ndarray:
    """[C, H, W] fp32 -> [C, 4*24*32] fp16 polyphased, unpadded."""
    v = x.reshape(C, SUB_H, 2, SUB_W, 2).transpose(0, 2, 4, 1, 3)
    return np.ascontiguousarray(v, dtype=np.float16).reshape(C, -1)


def _extract(bd: np.ndarray) -> np.ndarray:
    """[NPAIR, 128, PAIRW] fp16 compact band dump -> [441, 48, 64] fp32."""
    full = np.zeros((4, NYB, NXB, 128, WRH, WRW), np.float32)
    for s in range(4):
        for yb in range(NYB):
            n = ROWS[yb] * NCC
            rec = bd[s * NYB + yb].astype(np.float32)
            for xb in range(NXB):
                full[s, yb, xb, :, RLO[yb] : RLO[yb] + ROWS[yb],
                     CLO[xb] : CLO[xb] + NCC] = (
                    rec[:, xb * n : (xb + 1) * n].reshape(128, ROWS[yb], NCC)
                )
    bd = full.reshape(4, NYB, NXB, BH, BW, WRH, WRW)
    s = bd.strides
    # window of pixel (ry, rx) starts at band row ry, col rx: couple the
    # pixel strides with the window strides
    win = np.lib.stride_tricks.as_strided(
        bd,
        shape=(4, NYB, NXB, BH, BW, ND, ND),
        strides=(s[0], s[1], s[2], s[3] + s[5], s[4] + s[6], s[5], s[6]),
    )
    # [s, yb, xb, ry, rx, u, v] -> [u, v, yb, ry, xb, rx] per polyphase
    win = np.ascontiguousarray(win.transpose(0, 5, 6, 1, 3, 2, 4))
    out = np.empty((D, H, W), np.float32)
    ov = out.reshape(D, SUB_H, 2, SUB_W, 2)
    for sidx in range(4):
        sy, sx = sidx >> 1, sidx & 1
        ov[:, :, sy, :, sx] = win[sidx].reshape(D, SUB_H, SUB_W)
    return out


def kernel(input1: np.ndarray, input2: np.ndarray) -> np.ndarray:
    from concourse import bass_utils

    nc = _get_program()
    input1 = np.asarray(input1, dtype=np.float32)
    input2 = np.asarray(input2, dtype=np.float32)
    B = input1.shape[0]
    in_maps = [
        {"i1": _prep_i1(input1[b]), "i2": _prep_i2(input2[b])} for b in range(B)
    ]
    res = bass_utils.run_bass_kernel_spmd(nc, in_maps, core_ids=list(range(B)))
    return np.stack([_extract(r["od"]) for r in res.results])


# revision 44
# speedup vs baseline: 1.1010x; 1.1010x over previous
"""FlowNet correlation kernel for Trainium2 (8 NeuronCores, batch-parallel).

Problem: out[b, d, y, x] = (1/C) * sum_c i1[b,c,y,x] * pad(i2)[b,c,y+dy,x+dx]
  B=8, C=256, H=48, W=64, pad=20, displacements dy,dx in {-20..20 step 2}
  (21x21 = 441), output [8, 441, 48, 64] fp32.

Strategy (per core, one batch element):
  Displacement stride 2 => the problem splits into 4 independent polyphase
  subproblems (y-parity sy, x-parity sx), each a dense +-10 correlation on a
  24x32 quarter image. Block output pixels 8 sub-rows x 16 sub-cols
  (M = 128): each block's displacement band is a 28x36 window of the padded
  polyphase i2, but only the in-bounds part of it is ever computed — the
  matmuls are clipped to the real window rows AND columns (pad rows/cols of
  the band are statically zero and are filled in by the host). Per block
  that leaves real-rows x 26 band columns: 18x26=468 (fits one PSUM bank,
  single fp16 matmul per k-tile) for the top/bottom row-blocks, 24x26=624
  (split 12+12 over two banks) for the middle one. fp16 matmuls run at full
  PE rate with fp32 accumulation; the two 128-channel k-tiles accumulate in
  PSUM. The 1/C scale is applied during the PSUM->SBUF copy (fp16 band,
  split DVE/scalar), and the bands of each (s, yb) block pair are dumped
  with one contiguous big-packet DMA.

  Host-side prep (part of the sharding step, not device time): inputs are
  cast to fp16 and re-laid out per core — i1 pre-polyphased and pre-blocked
  [C, 4, 6, 128] so it DMAs directly into the stationary matmul layout, i2
  polyphased unpadded [C, 2, 2, 24, 32]. Input DMAs are chunked and ordered
  so the first block's operands land first, and the PE clock is pre-warmed
  with zero matmuls while inputs stream. The host scatters the compact
  bands back into zero-initialized full windows and extracts each pixel's
  21x21 displacement window (a strided view + copy) to assemble
  [441, 48, 64] fp32.

  History: an earlier design gathered the 441 displacement values per pixel
  on-device with diagonal-access-pattern DMAs — its 84-byte packets
  (64512/core) were DMA-packet-rate-bound. fp32 matmuls run at 1/4 PE rate
  (and float32r requires explicitly rounded producers), hence fp16. DMA
  queues sustain ~25 GB/s each (~400 GB/s/core aggregate) for packets
  >= ~1.5 KB, so all device DMAs are big contiguous runs and every byte
  shipped is real data.
"""

import numpy as np

C = 256
H, W = 48, 64
ND = 21                      # displacements per axis
D = ND * ND                  # 441
SUB_H, SUB_W = H // 2, W // 2          # 24, 32
BH, BW = 8, 16               # block = 8 x 16 output pixels (one polyphase)
WRH, WRW = BH + 20, BW + 20  # 28 x 36 full band window per block
NYB, NXB = SUB_H // BH, SUB_W // BW    # 3, 2
NCC = BW + 10                # 26 real band columns per block
ROWS = [18, 24, 18]          # real band rows per yb  (of 28)
RLO = [10, 2, 0]             # first real band row per yb
CLO = [10, 0]                # first real band column per xb
NPAIR = 4 * NYB              # 12 dumped (s, yb) pair records
PAIRW = 2 * max(ROWS) * NCC  # 1248 fp16 cols per pair record (max)

_CACHE = {}


def _build():
    import concourse.bacc as bacc
    import concourse.mybir as mybir
    from concourse.tile import TileContext

    f32 = mybir.dt.float32
    f16 = mybir.dt.float16

    nc = bacc.Bacc("TRN2", target_bir_lowering=False, debug=False)
    # i1: [C, s, blk, m] fp16, pre-polyphased/pre-blocked on host
    i1_t = nc.dram_tensor("i1", [C, 4 * NYB * NXB * 128], f16, kind="ExternalInput")
    # i2: [C, sy, sx, 24, 32] fp16, polyphased, unpadded
    i2_t = nc.dram_tensor("i2", [C, 4 * SUB_H * SUB_W], f16, kind="ExternalInput")
    od_t = nc.dram_tensor("od", [NPAIR, 128, PAIRW], f16, kind="ExternalOutput")

    inv_c = 1.0 / C

    with TileContext(nc) as tc:
        with (
            tc.tile_pool(name="inp", bufs=1) as inp_pool,
            tc.tile_pool(name="band", bufs=6) as band_pool,
            tc.tile_pool(name="ps", bufs=8, space="PSUM") as ps_pool,
        ):
            i1s_sb = [
                inp_pool.tile(
                    [128, 4 * NYB * NXB * 128], f16, name=f"i1k{k}", tag=f"i1k{k}"
                )
                for k in range(2)
            ]
            i2_sb = [
                inp_pool.tile(
                    [128, 4 * SUB_H * SUB_W], f16, name=f"i2k{k}", tag=f"i2k{k}"
                )
                for k in range(2)
            ]
            # [c, (sy sx), row, col] unpadded polyphase
            i2v = [
                t[:].rearrange("c (s r w) -> c s r w", s=4, r=SUB_H) for t in i2_sb
            ]

            # warm up the PE clock (HAM p-state ramps over ~3us of activity)
            # with zeros x zeros matmuls while the inputs stream in
            zt = inp_pool.tile([128, 504], f16, name="warmz", tag="warmz")
            nc.gpsimd.memset(zt[:], 0.0)
            wps = ps_pool.tile([128, 512], f32, name="ps")
            for _ in range(7):
                nc.tensor.matmul(
                    wps[:, 0:504], lhsT=zt[:, 0:128], rhs=zt[:], start=True,
                    stop=True,
                )

            # chunked input loads on the sync HWDGE queue, ordered to
            # unblock the first blocks first (scalar stays copy-only: it is
            # in the PSUM-drain path and must not fall behind the PE)
            for s in range(4):
                for k in range(2):
                    cs = slice(128 * k, 128 * (k + 1))
                    # the first block's k1 chunks issue on the scalar queue
                    # in parallel with sync's k0 chunks, cutting ~1us off
                    # the first matmul's data-ready time (scalar's 2 issues
                    # retire long before its first PSUM-drain copy)
                    q = nc.scalar if (s == 0 and k == 1) else nc.sync
                    if s % 2 == 0:  # i1 in two chunks per k (s01, s23)
                        q.dma_start(
                            out=i1s_sb[k][:, 1536 * (s // 2) : 1536 * (s // 2 + 1)],
                            in_=i1_t.ap()[cs, 1536 * (s // 2) : 1536 * (s // 2 + 1)],
                        )
                    q.dma_start(
                        out=i2v[k][:, s],
                        in_=i2_t.ap()[cs, 768 * s : 768 * (s + 1)],
                    )
            # pair dumps follow on the same queue: its head blocks on each
            # dump's copy-completion semaphores, so output streaming tracks
            # the compute pipeline pair by pair

            for s in range(4):
                for yb in range(NYB):
                    n = ROWS[yb] * NCC          # real band cols per block
                    # row spans per matmul: single for 18-row blocks, 12+12
                    # across two PSUM banks for the 24-row middle block
                    if ROWS[yb] <= 19:
                        spans = [(0, ROWS[yb])]
                    else:
                        h = ROWS[yb] // 2
                        spans = [(0, h), (h, ROWS[yb])]
                    r_base = BH * yb + RLO[yb] - 10  # first real i2 row
                    # the very last pair uses per-block band tiles + dumps so
                    # its first half streams out while the second computes
                    last = s == 3 and yb == NYB - 1
                    if last:
                        bands = [
                            band_pool.tile([128, PAIRW], f16, name="band")
                            for _ in range(NXB)
                        ]
                    else:
                        band2 = band_pool.tile([128, PAIRW], f16, name="band")
                    # one single-bank PSUM tile per (xb, span): bufs=8 keeps
                    # a full s-phase of banks in flight
                    pss = [
                        [ps_pool.tile([128, 512], f32, name="ps") for _ in spans]
                        for _ in range(NXB)
                    ]
                    # k outermost across the xb pair: all k0 matmuls run
                    # before any k1 data is needed, hiding the k1 input
                    # chunks' DMA-completion latency
                    for k in range(2):
                        for xb in range(NXB):
                            blk = s * NYB * NXB + yb * NXB + xb
                            c0 = CLO[xb] + BW * xb - 10  # first real i2 col
                            lhs = i1s_sb[k][:, 128 * blk : 128 * (blk + 1)]
                            for h, (lo, hi) in enumerate(spans):
                                rh = i2v[k][
                                    :,
                                    s,
                                    r_base + lo : r_base + hi,
                                    c0 : c0 + NCC,
                                ]
                                nc.tensor.matmul(
                                    pss[xb][h][:, 0 : NCC * (hi - lo)],
                                    lhsT=lhs,
                                    rhs=rh,
                                    start=(k == 0),
                                    stop=(k == 1),
                                )
                            if k == 1:
                                # compact to SBUF with the 1/C scale (fp16
                                # band); one copy per bank, split DVE/scalar
                                engs = (
                                    [nc.vector.tensor_scalar_mul, nc.scalar.mul]
                                    if xb == 0
                                    else [nc.scalar.mul, nc.vector.tensor_scalar_mul]
                                )
                                p = s * NYB + yb
                                for h, (lo, hi) in enumerate(spans):
                                    dst = (
                                        bands[xb][:, NCC * lo : NCC * hi]
                                        if last
                                        else band2[
                                            :,
                                            xb * n + NCC * lo : xb * n + NCC * hi,
                                        ]
                                    )
                                    engs[h % len(engs)](
                                        dst,
                                        pss[xb][h][:, 0 : NCC * (hi - lo)],
                                        inv_c,
                                    )
                                if last:
                                    nc.sync.dma_start(
                                        out=od_t.ap()[p][:, xb * n : (xb + 1) * n],
                                        in_=bands[xb][:, 0:n],
                                    )
                    if not last:
                        p = s * NYB + yb
                        nc.sync.dma_start(
                            out=od_t.ap()[p][:, 0 : 2 * n], in_=band2[:, 0 : 2 * n]
                        )

    nc.compile()
    return nc


def _get_program():
    if "nc" not in _CACHE:
        _CACHE["nc"] = _build()
    return _CACHE["nc"]


def _prep_i1(x: np.ndarray) -> np.ndarray:
    """[C, H, W] fp32 -> [C, 4*6*128] fp16 pre-polyphased + pre-blocked."""
    # [c, sy, sx, yb, xb, ry, rx] <- x[c, 16yb+2ry+sy, 32xb+2rx+sx]
    v = x.reshape(C, NYB, BH, 2, NXB, BW, 2)
    v = v.transpose(0, 3, 6, 1, 4, 2, 5)  # c, sy, sx, yb, xb, ry, rx
    return np.ascontiguousarray(v, dtype=np.float16).reshape(C, -1)


def _prep_i2(x: np.ndarray) -> np.ndarray:
    """[C, H, W] fp32 -> [C, 4*24*32] fp16 polyphased, unpadded."""
    v = x.reshape(C, SUB_H, 2, SUB_W, 2).transpose(0, 2, 4, 1, 3)
    return np.ascontiguousarray(v, dtype=np.float16).reshape(C, -1)


def _extract(bd: np.ndarray) -> np.ndarray:
    """[NPAIR, 128, PAIRW] fp16 compact band dump -> [441, 48, 64] fp32."""
    full = np.zeros((4, NYB, NXB, 128, WRH, WRW), np.float32)
    for s in range(4):
        for yb in range(NYB):
            n = ROWS[yb] * NCC
            rec = bd[s * NYB + yb].astype(np.float32)
            for xb in range(NXB):
                full[s, yb, xb, :, RLO[yb] : RLO[yb] + ROWS[yb],
                     CLO[xb] : CLO[xb] + NCC] = (
                    rec[:, xb * n : (xb + 1) * n].reshape(128, ROWS[yb], NCC)
                )
    bd = full.reshape(4, NYB, NXB, BH, BW, WRH, WRW)
    s = bd.strides
    # window of pixel (ry, rx) starts at band row ry, col rx: couple the
    # pixel strides with the window strides
    win = np.lib.stride_tricks.as_strided(
        bd,
        shape=(4, NYB, NXB, BH, BW, ND, ND),
        strides=(s[0], s[1], s[2], s[3] + s[5], s[4] + s[6], s[5], s[6]),
    )
    # [s, yb, xb, ry, rx, u, v] -> [u, v, yb, ry, xb, rx] per polyphase
    win = np.ascontiguousarray(win.transpose(0, 5, 6, 1, 3, 2, 4))
    out = np.empty((D, H, W), np.float32)
    ov = out.reshape(D, SUB_H, 2, SUB_W, 2)
    for sidx in range(4):
        sy, sx = sidx >> 1, sidx & 1
        ov[:, :, sy, :, sx] = win[sidx].reshape(D, SUB_H, SUB_W)
    return out


def kernel(input1: np.ndarray, input2: np.ndarray) -> np.ndarray:
    from concourse import bass_utils

    nc = _get_program()
    input1 = np.asarray(input1, dtype=np.float32)
    input2 = np.asarray(input2, dtype=np.float32)
    B = input1.shape[0]
    in_maps = [
        {"i1": _prep_i1(input1[b]), "i2": _prep_i2(input2[b])} for b in range(B)
    ]
    res = bass_utils.run_bass_kernel_spmd(nc, in_maps, core_ids=list(range(B)))
    return np.stack([_extract(r["od"]) for r in res.results])
